# revision 1
# baseline (speedup 1.0000x reference)
"""CTC loss (keras ctc_batch_cost semantics) on 8 Trainium2 NeuronCores.

Strategy (pure data parallelism, batch sharded 8 ways):
  - Device kernel runs the CTC forward DP in probability space with periodic
    max-rescaling (scale logs accumulated, summed at the end).
  - The per-batch gather q[b, t, ext[b, s]] is done on GPSIMD via ap_gather
    in a t-on-partitions layout (indices depend only on b, so they are
    shared across all 128 t-partitions — exactly the ap_gather contract),
    then PE transposes + ScalarE PSUM->SBUF copies regroup into the
    b-on-partitions layout the DP needs.
  - DP inner loop: 4 tensor_tensor ops per time step on [128, 4x33] tiles
    (batch 512 per core = 128 partitions x 4 groups), rescale every R steps.

Self-contained: hardcodes shapes from the problem spec.
"""

import numpy as np

# Problem dims (hardcoded per spec nn_CTCLayer_4518305595673)
B, T, C, L = 4096, 128, 96, 16
NCORES = 8
BC = B // NCORES            # 512 batches per core
S = 2 * L + 1               # 33 extended label positions
G4 = BC // 128              # 4 partition groups
BLANK = C - 1               # 95
EPS = 1e-7
R = 4                       # rescale every R time steps
GB = 64                     # batches per ap_gather call (HW-validated:
                            # fewer, larger ap_gather calls are much faster;
                            # 64 -> ~105us/iter vs 32 -> ~401us, 16 -> ~2.2ms)

_CACHE = {}


def _wc_pad(gb, s_len):
    """Wrapped idx columns per gather group, padded to 4-byte alignment."""
    wc = gb * s_len // 16
    return wc + (wc % 2)


def _build_program(bc=BC, t_len=T, c_dim=C, l_len=L, r_period=R, gb=GB,
                   v_gpsimd=False, repeat=1):
    """Build + compile the per-core Bass program."""
    import concourse.bacc as bacc
    import concourse.tile as tile
    from concourse import masks, mybir
    from contextlib import ExitStack

    s_len = 2 * l_len + 1
    sg = s_len + 2
    g4 = bc // 128
    nbg = bc // gb              # gather groups per core
    jpt = 128 // gb             # gather groups per 128-batch tile
    wc = gb * s_len // 16       # wrapped idx columns actually read
    wcp = _wc_pad(gb, s_len)    # stored (padded) columns
    ts = t_len * s_len
    resc_ts = sorted(set([t for t in range(1, t_len) if t % r_period == 0]
                         + [t_len - 1]))
    nsl = len(resc_ts)

    f32 = mybir.dt.float32
    i16 = mybir.dt.int16
    Alu = mybir.AluOpType
    Act = mybir.ActivationFunctionType
    Ax = mybir.AxisListType

    nc = bacc.Bacc("TRN2", target_bir_lowering=False, debug=False,
                   num_devices=NCORES)
    yp = nc.dram_tensor("yp", [bc, t_len, c_dim], f32, kind="ExternalInput")
    gidx = nc.dram_tensor("gidx", [128, nbg * wcp], i16, kind="ExternalInput")
    msk = nc.dram_tensor("mask", [128, g4 * s_len], f32, kind="ExternalInput")
    loss = nc.dram_tensor("loss", [bc, 1], f32, kind="ExternalOutput")

    with tile.TileContext(nc) as tc, ExitStack() as ctx:
        const_pool = ctx.enter_context(tc.tile_pool(name="const", bufs=1))
        load_pool = ctx.enter_context(tc.tile_pool(name="load", bufs=3))
        g_pool = ctx.enter_context(tc.tile_pool(name="gath", bufs=2))
        psum_pool = ctx.enter_context(
            tc.tile_pool(name="ps", bufs=4, space="PSUM"))
        big_pool = ctx.enter_context(tc.tile_pool(name="big", bufs=1))
        dp_pool = ctx.enter_context(tc.tile_pool(name="dp", bufs=1))

        ident = const_pool.tile([128, 128], f32)
        masks.make_identity(nc, ident[:])
        gidx_sb = const_pool.tile([128, nbg * wcp], i16)
        nc.sync.dma_start(gidx_sb[:], gidx.ap())
        mask_sb = const_pool.tile([128, g4 * s_len], f32)
        nc.sync.dma_start(mask_sb[:], msk.ap())
        mv = mask_sb[:].rearrange("p (g s) -> p g s", g=g4)

        def body():
            qe = big_pool.tile([128, g4 * ts], f32, tag="qe")
            qev = qe[:].rearrange("p (g t s) -> p g t s", g=g4, t=t_len)

            # ---- gather + regroup phase ----
            for bt in range(g4):
                g_tile = g_pool.tile([128, 128 * s_len], f32, tag="gt")
                for jj in range(jpt):
                    j = bt * jpt + jj
                    l_tile = load_pool.tile([128, gb * c_dim], f32, tag="ld")
                    src = (yp.ap()[gb * j:gb * (j + 1)]
                           .rearrange("b t c -> t b c"))
                    dst = l_tile[:].rearrange("p (b c) -> p b c", b=gb)
                    nc.sync.dma_start(dst, src)
                    nc.gpsimd.ap_gather(
                        out_ap=g_tile[:, jj * gb * s_len:(jj + 1) * gb * s_len],
                        in_ap=l_tile[:],
                        idxs_ap=gidx_sb[:, j * wcp:j * wcp + wc],
                        channels=128,
                        num_elems=gb * c_dim,
                        d=1,
                        num_idxs=gb * s_len,
                    )
                gv = g_tile[:].rearrange("p (j i s) -> p j i s", j=jpt, i=gb)
                for s in range(s_len):
                    pt = psum_pool.tile([128, 128], f32, tag="pt")
                    nc.tensor.transpose(pt[:], gv[:, :, :, s], ident[:])
                    nc.scalar.activation(qev[:, bt, :, s], pt[:], Act.Copy,
                                         bias=EPS)

            # ---- DP phase ----
            alpha_a = dp_pool.tile([128, g4 * sg], f32, tag="alpha_a")
            alpha_b = dp_pool.tile([128, g4 * sg], f32, tag="alpha_b")
            a_tiles = [alpha_a, alpha_b]
            for a in a_tiles:
                nc.vector.memset(a[:], 0.0)
            av = [a[:].rearrange("p (g s) -> p g s", g=g4) for a in a_tiles]

            u_t = dp_pool.tile([128, g4 * s_len], f32, tag="u_t")
            v_t = dp_pool.tile([128, g4 * s_len], f32, tag="v_t")
            uv = u_t[:].rearrange("p (g s) -> p g s", g=g4)
            vv = v_t[:].rearrange("p (g s) -> p g s", g=g4)

            scl = dp_pool.tile([128, g4 * nsl], f32, tag="scl")
            sclv = scl[:].rearrange("p (g n) -> p g n", g=g4)
            rec = dp_pool.tile([128, g4], f32, tag="rec")

            # t = 0 init: alpha[s=0,1] = q'[0, s], rest 0
            nc.vector.tensor_copy(av[0][:, :, 2:4], qev[:, :, 0, 0:2])

            cur = 0
            for t in range(1, t_len):
                prev, nxt = av[cur], av[1 - cur]
                nc.vector.tensor_tensor(uv[:, :, :], prev[:, :, 2:2 + s_len],
                                        prev[:, :, 1:1 + s_len], op=Alu.add)
                veng = nc.gpsimd if v_gpsimd else nc.vector
                veng.tensor_tensor(vv[:, :, :], prev[:, :, 0:s_len],
                                   mv[:, :, :], op=Alu.mult)
                nc.vector.tensor_tensor(uv[:, :, :], uv[:, :, :], vv[:, :, :],
                                        op=Alu.add)
                nc.vector.tensor_tensor(nxt[:, :, 2:2 + s_len], uv[:, :, :],
                                        qev[:, :, t, :], op=Alu.mult)
                if t in resc_ts:
                    slot = resc_ts.index(t)
                    nc.vector.tensor_reduce(sclv[:, :, slot],
                                            nxt[:, :, 2:2 + s_len],
                                            axis=Ax.X, op=Alu.max)
                    nc.vector.reciprocal(rec[:], sclv[:, :, slot])
                    bb = rec[:].unsqueeze(2).broadcast_to((128, g4, s_len))
                    nc.vector.tensor_tensor(nxt[:, :, 2:2 + s_len],
                                            nxt[:, :, 2:2 + s_len], bb,
                                            op=Alu.mult)
                cur = 1 - cur

            # ---- epilogue ----
            lg = dp_pool.tile([128, g4 * nsl], f32, tag="lg")
            nc.scalar.activation(lg[:], scl[:], Act.Ln)
            lsum = dp_pool.tile([128, g4], f32, tag="lsum")
            nc.vector.tensor_reduce(lsum[:],
                                    lg[:].rearrange("p (g n) -> p g n", g=g4),
                                    axis=Ax.X, op=Alu.add)
            tail = dp_pool.tile([128, g4], f32, tag="tail")
            fin = av[cur]
            nc.vector.tensor_tensor(tail[:], fin[:, :, sg - 2],
                                    fin[:, :, sg - 1], op=Alu.add)
            ltail = dp_pool.tile([128, g4], f32, tag="ltail")
            nc.scalar.activation(ltail[:], tail[:], Act.Ln)
            tot = dp_pool.tile([128, g4], f32, tag="tot")
            nc.vector.tensor_tensor(tot[:], lsum[:], ltail[:], op=Alu.add)
            loss_sb = dp_pool.tile([128, g4], f32, tag="loss_sb")
            nc.vector.tensor_scalar_mul(loss_sb[:], tot[:], -1.0)
            nc.sync.dma_start(
                loss.ap().rearrange("(g p) one -> p (g one)", p=128),
                loss_sb[:])

        for _rep in range(repeat):
            body()

    nc.compile()
    return nc


def _host_prep(y_true, y_pred, bc=BC, gb=GB, s_len=S):
    """Shard + build index/mask tensors. Returns in_maps list."""
    y_true = np.asarray(y_true).astype(np.int64)
    y_pred = np.ascontiguousarray(np.asarray(y_pred), dtype=np.float32)
    ncores = y_pred.shape[0] // bc
    g4 = bc // 128
    nbg = bc // gb
    wc = gb * s_len // 16
    wcp = _wc_pad(gb, s_len)
    ext = np.full((y_true.shape[0], s_len), BLANK, dtype=np.int64)
    ext[:, 1::2] = y_true
    mask_full = np.zeros((ext.shape[0], s_len), dtype=np.float32)
    mask_full[:, 2:] = ((ext[:, 2:] != ext[:, :-2])
                        & (ext[:, 2:] != BLANK)).astype(np.float32)

    in_maps = []
    for cid in range(ncores):
        b0 = cid * bc
        yp_c = y_pred[b0:b0 + bc]
        ext_c = ext[b0:b0 + bc]
        # gather indices: per gb-batch group j, idxlist[(i, s)] = i*C + ext;
        # wrapped in 16 partitions: stored[p%16, w] = idxlist[w*16 + p%16].
        # Each group's slice is padded to an even column count so every
        # slice base is 4-byte aligned (ap_gather HW requirement).
        gidx_c = np.zeros((128, nbg * wcp), dtype=np.int16)
        for j in range(nbg):
            idxlist = (np.arange(gb, dtype=np.int16)[:, None] * C
                       + ext_c[gb * j:gb * (j + 1)].astype(np.int16))
            wrapped = idxlist.reshape(-1).reshape(wc, 16).T  # [p16, w]
            gidx_c[:, j * wcp:j * wcp + wc] = np.tile(wrapped, (8, 1))
        m = mask_full[b0:b0 + bc].reshape(g4, 128, s_len).transpose(1, 0, 2)
        mask_c = np.ascontiguousarray(m.reshape(128, g4 * s_len))
        in_maps.append({"yp": yp_c, "gidx": gidx_c, "mask": mask_c})
    return in_maps


def get_program(repeat=1):
    key = ("nc", repeat)
    if key not in _CACHE:
        _CACHE[key] = _build_program(repeat=repeat)
    return _CACHE[key]


def kernel(y_true, y_pred):
    from concourse import bass_utils
    nc = get_program()
    in_maps = _host_prep(y_true, y_pred)
    res = bass_utils.run_bass_kernel_spmd(nc, in_maps,
                                          core_ids=list(range(NCORES)))
    out = np.concatenate([res.results[c]["loss"] for c in range(NCORES)],
                         axis=0)
    return out.astype(np.float32)



# revision 2
# speedup vs baseline: 17.0522x; 17.0522x over previous
"""CTC loss on 8 Trainium2 NeuronCores - fb split, fp32 DP, bf16 gather,
exponent-split logs.

Architecture (on top of kernel3's dma_gather + ScalarE expansion):
  - Each core runs the forward DP on t in [0,64) AND the time-reversed
    backward DP on t in [64,128) for its 512 batches SIMULTANEOUSLY:
    the backward recurrence in reversed state order (sigma = S-1-s) has
    exactly the forward update shape, so alpha (4 segments) and gamma'
    (4 segments) live in one [128, 8*36] tile updated by the SAME 4
    tensor_tensor ops at FD=272 - halving the serial step count from 127
    to 63 while amortizing the DVE per-op overhead over 2x the elements.
  - The gather is unchanged (17 full t-rows per batch); the expansion
    writes the backward segments with time+label order reversed via
    negative-stride APs (free on ScalarE).
  - Combine: one u-step on the backward segments + a reversed-operand
    dot with alpha, then logs.
"""

import numpy as np

B, T, C, L = 4096, 128, 96, 16
NCORES = 8
BC = B // NCORES
S = 2 * L + 1               # 33
SP = S + 1                  # 34
G4 = BC // 128              # 4 batch groups
NSEG = 2 * G4               # 8 segments (4 fwd + 4 bwd)
TL = T // 2                 # 64 local time steps
BLANK = C - 1
EPS = 1e-7
R = 8
NI = L + 1
GPC = 2
NCALL = G4 // GPC
NIDX = GPC * NI * 128
WPC = NIDX // 16

_CACHE = {}


def _build_program(repeat=1):
    import concourse.bacc as bacc
    import concourse.tile as tile
    from concourse import mybir
    from contextlib import ExitStack

    g4 = G4
    sg = 36
    resc_ts = sorted(set([t for t in range(1, TL) if t % R == 0] + [TL - 1]))
    nsl = len(resc_ts)

    f32 = mybir.dt.float32
    bf16 = mybir.dt.bfloat16
    i16 = mybir.dt.int16
    i32 = mybir.dt.int32
    LN2 = float(np.log(2.0))
    Alu = mybir.AluOpType
    Act = mybir.ActivationFunctionType
    Ax = mybir.AxisListType

    nc = bacc.Bacc("TRN2", target_bir_lowering=False, debug=False,
                   num_devices=NCORES)
    ypt = nc.dram_tensor("ypt", [BC * C, T], bf16, kind="ExternalInput")
    gidx = nc.dram_tensor("gidx", [128, NCALL * WPC], i16,
                          kind="ExternalInput")
    msk = nc.dram_tensor("mask", [128, NSEG * SP], f32,
                         kind="ExternalInput")
    loss = nc.dram_tensor("loss", [BC, 1], f32, kind="ExternalOutput")

    with tile.TileContext(nc) as tc, ExitStack() as ctx:
        const_pool = ctx.enter_context(tc.tile_pool(name="const", bufs=1))
        g_pool = ctx.enter_context(tc.tile_pool(name="gath", bufs=2))
        big_pool = ctx.enter_context(tc.tile_pool(name="big", bufs=2))
        dp_pool = ctx.enter_context(tc.tile_pool(name="dp", bufs=1))

        gidx_sb = const_pool.tile([128, NCALL * WPC], i16)
        nc.sync.dma_start(gidx_sb[:], gidx.ap())
        mask_sb = const_pool.tile([128, NSEG * SP], f32)
        nc.sync.dma_start(mask_sb[:], msk.ap())
        mv = mask_sb[:].rearrange("p (g s) -> p g s", g=NSEG)

        def body():
            # ---- gather: 17 full t-rows per batch ----
            q17 = g_pool.tile([128, g4 * NI * T], bf16, tag="q17")
            q17v = q17[:].rearrange("p (g i t) -> p g i t", g=g4, i=NI)
            for call in range(NCALL):
                rows = GPC * 128 * C
                nc.gpsimd.dma_gather(
                    out_ap=q17[:].rearrange("p (n t) -> p n t", t=T)
                    [:, call * GPC * NI:(call + 1) * GPC * NI, :],
                    in_ap=ypt.ap()[call * rows:(call + 1) * rows],
                    idxs_ap=gidx_sb[:, call * WPC:(call + 1) * WPC],
                    num_idxs=NIDX,
                    num_idxs_reg=NIDX,
                    elem_size=T,
                    single_packet=False,
                )

            # ---- expansion: qe[b_p, seg8, tl64, s34] bf16 with EPS ----
            qe = big_pool.tile([128, NSEG * TL * SP], f32, tag="qe")
            qev = qe[:].rearrange("p (g t s) -> p g t s", g=NSEG, t=TL)
            q17t = q17[:].rearrange("p (g i t) -> p g t i", g=g4, i=NI)
            nc.vector.memset(qev[:, :, :, S], 0.0)
            for g in range(g4):
                # forward segments: real t = local t'
                nc.scalar.activation(
                    qev[:, g, :, 1:1 + 2 * L].rearrange(
                        "p t (l two) -> p t l two", two=2)[:, :, :, 0],
                    q17t[:, g, 0:TL, 0:L], Act.Copy, bias=EPS)
                nc.scalar.activation(
                    qev[:, g, :, 0:SP].rearrange(
                        "p t (k two) -> p t k two", two=2)[:, :, :, 0],
                    q17v[:, g, L, 0:TL].unsqueeze(2).broadcast_to(
                        (128, TL, L + 1)), Act.Copy, bias=EPS)
                # backward segments: real t = 127 - t', labels reversed
                nc.scalar.activation(
                    qev[:, g4 + g, :, 1:1 + 2 * L].rearrange(
                        "p t (l two) -> p t l two", two=2)[:, :, :, 0],
                    q17t[:, g, TL:T, 0:L][:, ::-1, ::-1], Act.Copy, bias=EPS)
                nc.scalar.activation(
                    qev[:, g4 + g, :, 0:SP].rearrange(
                        "p t (k two) -> p t k two", two=2)[:, :, :, 0],
                    q17v[:, g, L, TL:T][:, ::-1].unsqueeze(2).broadcast_to(
                        (128, TL, L + 1)), Act.Copy, bias=EPS)

            # ---- DP phase: 63 steps over 8 segments ----
            alpha_a = dp_pool.tile([128, NSEG * sg], f32, tag="alpha_a")
            alpha_b = dp_pool.tile([128, NSEG * sg], f32, tag="alpha_b")
            a_tiles = [alpha_a, alpha_b]
            for a in a_tiles:
                nc.vector.memset(a[:], 0.0)
            av = [a[:].rearrange("p (g s) -> p g s", g=NSEG) for a in a_tiles]

            u_t = dp_pool.tile([128, NSEG * SP], f32, tag="u_t")
            v_t = dp_pool.tile([128, NSEG * SP], f32, tag="v_t")
            uv = u_t[:].rearrange("p (g s) -> p g s", g=NSEG)
            vv = v_t[:].rearrange("p (g s) -> p g s", g=NSEG)

            scl = dp_pool.tile([128, NSEG * nsl], f32, tag="scl")
            sclv = scl[:].rearrange("p (g n) -> p g n", g=NSEG)
            rec = dp_pool.tile([128, NSEG], f32, tag="rec")

            # unified init: fwd alpha_0 and bwd gamma'_127 both = q[t'=0, 0:2]
            nc.vector.tensor_copy(av[0][:, :, 2:4], qev[:, :, 0, 0:2])

            cur = 0
            for t in range(1, TL):
                prev, nxt = av[cur], av[1 - cur]
                nc.vector.tensor_tensor(uv[:, :, :], prev[:, :, 2:2 + SP],
                                        prev[:, :, 1:1 + SP], op=Alu.add)
                nc.vector.tensor_tensor(vv[:, :, :], prev[:, :, 0:SP],
                                        mv[:, :, :], op=Alu.mult)
                nc.vector.tensor_tensor(uv[:, :, :], uv[:, :, :], vv[:, :, :],
                                        op=Alu.add)
                nc.vector.tensor_tensor(nxt[:, :, 2:2 + SP], uv[:, :, :],
                                        qev[:, :, t, 0:SP], op=Alu.mult)
                if t in resc_ts:
                    slot = resc_ts.index(t)
                    nc.vector.tensor_reduce(sclv[:, :, slot],
                                            nxt[:, :, 2:2 + SP],
                                            axis=Ax.X, op=Alu.max)
                    nc.vector.reciprocal(rec[:], sclv[:, :, slot])
                    bb = rec[:].unsqueeze(2).broadcast_to((128, NSEG, SP))
                    nc.vector.tensor_tensor(nxt[:, :, 2:2 + SP],
                                            nxt[:, :, 2:2 + SP], bb,
                                            op=Alu.mult)
                cur = 1 - cur

            # ---- combine: beta u-step on bwd segments, reversed dot ----
            fin = av[cur]
            ub = dp_pool.tile([128, g4 * SP], f32, tag="ub")
            vb = dp_pool.tile([128, g4 * SP], f32, tag="vb")
            ubv = ub[:].rearrange("p (g s) -> p g s", g=g4)
            vbv = vb[:].rearrange("p (g s) -> p g s", g=g4)
            nc.vector.tensor_tensor(ubv[:, :, :], fin[:, g4:NSEG, 2:2 + SP],
                                    fin[:, g4:NSEG, 1:1 + SP], op=Alu.add)
            nc.vector.tensor_tensor(vbv[:, :, :], fin[:, g4:NSEG, 0:SP],
                                    mv[:, g4:NSEG, :], op=Alu.mult)
            nc.vector.tensor_tensor(ubv[:, :, :], ubv[:, :, :], vbv[:, :, :],
                                    op=Alu.add)
            # w[sigma] = alpha[S-1-sigma] * beta'[sigma]
            w = dp_pool.tile([128, g4 * S], f32, tag="w")
            wv = w[:].rearrange("p (g s) -> p g s", g=g4)
            nc.vector.tensor_tensor(wv[:, :, :],
                                    fin[:, 0:g4, 2:2 + S][:, :, ::-1],
                                    ubv[:, :, 0:S], op=Alu.mult)
            dsum = dp_pool.tile([128, g4], f32, tag="dsum")
            nc.vector.tensor_reduce(dsum[:], wv[:, :, :], axis=Ax.X,
                                    op=Alu.add)

            # ---- epilogue: logs via exponent split ----
            # (HW Ln loses absolute accuracy below ~2^-60; feed it only
            #  mantissas in [1,2) and add the exponent*ln2 separately.)
            def ln_split(out_ap, x_ap, n, name):
                xi = x_ap.bitcast(i32)
                ei = dp_pool.tile([128, n], i32, tag=name + "_ei")
                nc.vector.tensor_scalar(ei[:], xi, 23, None,
                                        op0=Alu.logical_shift_right)
                mi = dp_pool.tile([128, n], i32, tag=name + "_mi")
                nc.vector.tensor_scalar(mi[:], xi, 0x007FFFFF, 0x3F800000,
                                        op0=Alu.bitwise_and,
                                        op1=Alu.bitwise_or)
                lnm = dp_pool.tile([128, n], f32, tag=name + "_lnm")
                nc.scalar.activation(lnm[:], mi[:].bitcast(f32), Act.Ln)
                ef = dp_pool.tile([128, n], f32, tag=name + "_ef")
                nc.vector.tensor_copy(ef[:], ei[:])
                nc.vector.tensor_scalar(ef[:], ef[:], LN2, -127.0 * LN2,
                                        op0=Alu.mult, op1=Alu.add)
                nc.vector.tensor_tensor(out_ap, lnm[:], ef[:], op=Alu.add)

            lg = dp_pool.tile([128, NSEG * nsl], f32, tag="lg")
            ln_split(lg[:], scl[:], NSEG * nsl, "lg")
            ls8 = dp_pool.tile([128, NSEG], f32, tag="ls8")
            nc.vector.tensor_reduce(ls8[:],
                                    lg[:].rearrange("p (g n) -> p g n",
                                                    g=NSEG),
                                    axis=Ax.X, op=Alu.add)
            lsum = dp_pool.tile([128, g4], f32, tag="lsum")
            nc.vector.tensor_tensor(lsum[:], ls8[:, 0:g4], ls8[:, g4:NSEG],
                                    op=Alu.add)
            nc.vector.tensor_scalar_max(dsum[:], dsum[:], 1.2e-38)
            ldot = dp_pool.tile([128, g4], f32, tag="ldot")
            ln_split(ldot[:], dsum[:], g4, "ldot")
            tot = dp_pool.tile([128, g4], f32, tag="tot")
            nc.vector.tensor_tensor(tot[:], lsum[:], ldot[:], op=Alu.add)
            loss_sb = dp_pool.tile([128, g4], f32, tag="loss_sb")
            nc.vector.tensor_scalar_mul(loss_sb[:], tot[:], -1.0)
            nc.sync.dma_start(
                loss.ap().rearrange("(g p) one -> p (g one)", p=128),
                loss_sb[:])

        for _rep in range(repeat):
            body()

    nc.compile()
    return nc


def _host_prep(y_true, y_pred):
    import ml_dtypes
    y_true = np.asarray(y_true).astype(np.int64)
    y_pred = np.asarray(y_pred).astype(np.float32)
    ncores = y_pred.shape[0] // BC

    ext = np.full((y_true.shape[0], S), BLANK, dtype=np.int64)
    ext[:, 1::2] = y_true
    m_fwd = np.zeros((ext.shape[0], SP), dtype=np.float32)
    m_fwd[:, 2:S] = ((ext[:, 2:] != ext[:, :-2])
                     & (ext[:, 2:] != BLANK)).astype(np.float32)
    # backward mask in sigma space: m'[sig] = m[34 - sig] for sig in [2, 33)
    m_bwd = np.zeros((ext.shape[0], SP), dtype=np.float32)
    sig = np.arange(2, S)
    m_bwd[:, sig] = m_fwd[:, 34 - sig]

    cols = np.concatenate(
        [y_true.astype(np.int32), np.full((y_true.shape[0], 1), BLANK,
                                          dtype=np.int32)], axis=1)

    in_maps = []
    for cid in range(ncores):
        b0 = cid * BC
        ypc = y_pred[b0:b0 + BC]
        ypt = np.ascontiguousarray(
            ypc.transpose(0, 2, 1).reshape(BC * C, T)).astype(
                ml_dtypes.bfloat16)

        gidx_c = np.zeros((128, NCALL * WPC), dtype=np.int16)
        for call in range(NCALL):
            jj = np.arange(GPC * NI)
            g_local = jj // NI
            i = jj % NI
            p = np.arange(128)
            b_global = b0 + (call * GPC + g_local)[:, None] * 128 + p[None, :]
            local_row = (g_local[:, None] * 128 + p[None, :]) * C
            idx = (local_row + cols[b_global, i[:, None]]).astype(np.int16)
            wrapped = idx.reshape(-1).reshape(NIDX // 16, 16).T
            gidx_c[:, call * WPC:(call + 1) * WPC] = np.tile(wrapped, (8, 1))

        def seg(mfull):
            m = mfull[b0:b0 + BC].reshape(G4, 128, SP).transpose(1, 0, 2)
            return m.reshape(128, G4 * SP)
        mask_c = np.ascontiguousarray(
            np.concatenate([seg(m_fwd), seg(m_bwd)], axis=1))
        in_maps.append({"ypt": ypt, "gidx": gidx_c, "mask": mask_c})
    return in_maps


def get_program(repeat=1):
    key = ("nc", repeat)
    if key not in _CACHE:
        _CACHE[key] = _build_program(repeat=repeat)
    return _CACHE[key]


def kernel(y_true, y_pred):
    from concourse import bass_utils
    nc = get_program()
    in_maps = _host_prep(y_true, y_pred)
    res = bass_utils.run_bass_kernel_spmd(nc, in_maps,
                                          core_ids=list(range(NCORES)))
    out = np.concatenate([res.results[c]["loss"] for c in range(NCORES)],
                         axis=0)
    return out.astype(np.float32)
